# revision 1
# baseline (speedup 1.0000x reference)
"""Hard-triplet miner for Trainium2, 8-core SPMD.

Per core: compute a [1024, 8192] strip of the Gram matrix G = x_norm @ x_norm.T
on the PE, then per 128-row tile build w = G - 2*[same_label] in one fused
DVE tensor_tensor_reduce pass (per-column-tile maxima as a byproduct).
Since sqrt/constant shifts are monotonic: hardest negative = argmax_j w,
hardest positive = argmin_j w.  Index extraction: one max_index pass whose
in_max carries BOTH the row max and the row min (max_index is a value
matcher).  keep = thresholds on the two extremes.
"""

import numpy as np

import concourse.bacc as bacc
import concourse.bass as bass
import concourse.mybir as mybir
import concourse.tile as tile
from concourse import masks
from concourse.bass_utils import run_bass_kernel_spmd

F32 = mybir.dt.float32
BF16 = mybir.dt.bfloat16
U32 = mybir.dt.uint32

N = 8192          # total rows
D = 128           # embed dim
NCORES = 8
STRIP = N // NCORES       # 1024 anchor rows per core
RT = STRIP // 128         # 8 row-tiles per core
CT_W = 1024               # column-tile width for psum/ttr
CT = N // CT_W            # 8 column tiles
NEG_INIT = -1.0e30
PAD_VAL = 3.0e38


def build_program(k_repeat: int = 1, use_for_i: bool = False, n: int = N,
                  strip: int = STRIP, debug_level: int = 0,
                  mask_f32: bool = True):
    """Build the SPMD program (identical on all cores).  n/strip shrinkable
    for simulator validation."""
    rt_n = strip // 128
    ct_n = n // CT_W if n >= CT_W else 1
    ct_w = min(CT_W, n)
    t_full = n // 128

    nc = bacc.Bacc("TRN2", target_bir_lowering=False, debug=False,
                   num_devices=NCORES)

    x_full = nc.dram_tensor("x_full", [n, D], F32, kind="ExternalInput")
    x_strip = nc.dram_tensor("x_strip", [strip, D], F32, kind="ExternalInput")
    lab_full = nc.dram_tensor("lab_full", [1, n], F32, kind="ExternalInput")
    lab_strip = nc.dram_tensor("lab_strip", [128, rt_n], F32,
                               kind="ExternalInput")
    neg_out = nc.dram_tensor("neg_out", [128, rt_n], U32, kind="ExternalOutput")
    pos_out = nc.dram_tensor("pos_out", [128, rt_n], U32, kind="ExternalOutput")
    keep_out = nc.dram_tensor("keep_out", [128, rt_n], F32,
                              kind="ExternalOutput")

    with tile.TileContext(nc) as tc:
        with (
            tc.tile_pool(name="persist", bufs=1) as persist,
            tc.tile_pool(name="rowp", bufs=3) as rowp,
            tc.tile_pool(name="maskp", bufs=1) as maskp,
            tc.tile_pool(name="nescp", bufs=1) as nescp,
            tc.tile_pool(name="wp", bufs=2) as wp,
            tc.tile_pool(name="smalls", bufs=4) as smalls,
            tc.tile_pool(name="psum_pro", bufs=2,
                         space=bass.MemorySpace.PSUM) as psum_pro,
            tc.tile_pool(name="psum_main", bufs=3,
                         space=bass.MemorySpace.PSUM) as psum_main,
        ):
            ident = persist.tile([128, 128], F32)
            masks.make_identity(nc, ident[:])

            xT = persist.tile([128, n], F32, tag="xT")
            xsT = persist.tile([128, strip], F32, tag="xsT")
            labrep = persist.tile([128, n], BF16, tag="labrep")
            labsT = persist.tile([128, rt_n], F32, tag="labsT")
            ones1 = persist.tile([1, 128], F32, tag="ones1")
            nc.gpsimd.memset(ones1[:], 1.0)
            lab1 = persist.tile([1, n], F32, tag="lab1")

            nc.sync.dma_start(lab1[:], lab_full[:])
            nc.sync.dma_start(labsT[:], lab_strip[:])

            bias2 = persist.tile([128, 1], F32, tag="bias2")
            nc.gpsimd.memset(bias2[:], 2.0)
            bias09 = persist.tile([128, 1], F32, tag="bias09")
            nc.gpsimd.memset(bias09[:], 0.9)
            biasm09 = persist.tile([128, 1], F32, tag="biasm09")
            nc.gpsimd.memset(biasm09[:], -0.9)
            bias0 = persist.tile([128, 1], F32, tag="bias0")
            nc.gpsimd.memset(bias0[:], 0.0)

            # --- normalize + transpose: build xT (all rows) and xsT (strip) ---
            def norm_transpose(dst, src_dram, tiles):
                for t in range(tiles):
                    row = rowp.tile([128, D], F32, tag="row")
                    nc.sync.dma_start(row[:], src_dram[t * 128:(t + 1) * 128, :])
                    sq = rowp.tile([128, D], F32, tag="sq")
                    ssq = smalls.tile([128, 1], F32, tag="ssq")
                    nc.scalar.activation(sq[:], row[:],
                                         mybir.ActivationFunctionType.Square,
                                         bias=bias0[:], accum_out=ssq[:])
                    nrm = smalls.tile([128, 1], F32, tag="nrm")
                    nc.scalar.activation(nrm[:], ssq[:],
                                         mybir.ActivationFunctionType.Sqrt,
                                         bias=bias0[:])
                    rin = smalls.tile([128, 1], F32, tag="rin")
                    nc.vector.reciprocal(rin[:], nrm[:])
                    xn = rowp.tile([128, D], F32, tag="xn")
                    nc.vector.tensor_scalar_mul(xn[:], row[:], rin[:])
                    pt = psum_pro.tile([128, 512], F32, tag="ppro")
                    nc.tensor.transpose(pt[:, 0:128], xn[:], ident[:])
                    nc.scalar.activation(dst[:, t * 128:(t + 1) * 128],
                                         pt[:, 0:128],
                                         mybir.ActivationFunctionType.Copy)

            norm_transpose(xT, x_full, t_full)
            norm_transpose(xsT, x_strip, rt_n)

            # --- replicate labels across partitions (matmul broadcast) ---
            for c in range(n // 512):
                pl = psum_pro.tile([128, 512], F32, tag="ppro")
                nc.tensor.matmul(pl[:], ones1[:], lab1[:, c * 512:(c + 1) * 512])
                nc.scalar.activation(labrep[:, c * 512:(c + 1) * 512], pl[:],
                                     mybir.ActivationFunctionType.Copy)

            labsTm2 = persist.tile([128, rt_n], F32, tag="labsTm2")
            nc.vector.tensor_scalar_mul(labsTm2[:], labsT[:], -2.0)

            neg_stage = persist.tile([128, rt_n], U32, tag="neg_stage")
            pos_stage = persist.tile([128, rt_n], U32, tag="pos_stage")
            keep_stage = persist.tile([128, rt_n], F32, tag="keep_stage")

            def main_body():
                for rt in range(rt_n):
                    if debug_level >= 3:
                        nc.vector.memset(neg_stage[:, rt:rt + 1], 0)
                        nc.vector.memset(pos_stage[:, rt:rt + 1], 0)
                        nc.vector.memset(keep_stage[:, rt:rt + 1], 0)
                        continue
                    # nesc = |2*lab_j - 2*lab_i|; eqsc = relu(2 - nesc)
                    # => 2.0 where labels equal, 0 where different
                    nesc = nescp.tile([128, n], BF16, tag="nesc")
                    nc.scalar.activation(nesc[:], labrep[:],
                                         mybir.ActivationFunctionType.Abs,
                                         scale=2.0,
                                         bias=labsTm2[:, rt:rt + 1])
                    eqsc = maskp.tile([128, n], F32 if mask_f32 else BF16,
                                      tag="eqsc")
                    nc.scalar.activation(eqsc[:], nesc[:],
                                         mybir.ActivationFunctionType.Relu,
                                         scale=-1.0, bias=bias2[:])
                    w = wp.tile([128, n], F32, tag="w")
                    slots = smalls.tile([128, 8], F32, tag="slots")
                    if debug_level == 2:
                        nc.vector.memset(slots[:], 0)
                        nc.vector.tensor_copy(neg_stage[:, rt:rt + 1],
                                              slots[:, 0:1])
                        nc.vector.tensor_copy(pos_stage[:, rt:rt + 1],
                                              slots[:, 1:2])
                        nc.vector.tensor_copy(keep_stage[:, rt:rt + 1],
                                              slots[:, 2:3])
                        continue
                    for ct in range(ct_n):
                        ps = psum_main.tile([128, ct_w], F32, tag="ps")
                        for h in range(ct_w // 512):
                            lo = ct * ct_w + h * 512
                            nc.tensor.matmul(
                                ps[:, h * 512:(h + 1) * 512],
                                xsT[:, rt * 128:(rt + 1) * 128],
                                xT[:, lo:lo + 512])
                        if debug_level == 4:
                            nc.scalar.activation(
                                w[:, ct * ct_w:(ct + 1) * ct_w], ps[:],
                                mybir.ActivationFunctionType.Copy)
                            continue
                        nc.vector.tensor_tensor(
                            w[:, ct * ct_w:(ct + 1) * ct_w], ps[:],
                            eqsc[:, ct * ct_w:(ct + 1) * ct_w],
                            mybir.AluOpType.subtract)
                    if debug_level >= 1:
                        nc.vector.memset(slots[:], 0)
                        nc.vector.tensor_copy(neg_stage[:, rt:rt + 1],
                                              slots[:, 0:1])
                        nc.vector.tensor_copy(pos_stage[:, rt:rt + 1],
                                              slots[:, 1:2])
                        nc.vector.tensor_copy(keep_stage[:, rt:rt + 1],
                                              slots[:, 2:3])
                        continue
                    # ---- extraction ----
                    top8 = smalls.tile([128, 1], F32, tag="top8")
                    nc.vector.tensor_reduce(top8[:], w[:], mybir.AxisListType.X,
                                            mybir.AluOpType.max)
                    gmin = smalls.tile([128, 1], F32, tag="gmin")
                    nc.vector.tensor_reduce(gmin[:], w[:], mybir.AxisListType.X,
                                            mybir.AluOpType.min)
                    inmax = smalls.tile([128, 8], F32, tag="inmax")
                    nc.vector.memset(inmax[:], PAD_VAL)
                    nc.vector.tensor_copy(inmax[:, 0:1], top8[:, 0:1])
                    nc.vector.tensor_copy(inmax[:, 1:2], gmin[:])
                    idx8 = smalls.tile([128, 8], U32, tag="idx8")
                    nc.vector.max_index(idx8[:], inmax[:], w[:])
                    nc.vector.tensor_copy(neg_stage[:, rt:rt + 1], idx8[:, 0:1])
                    nc.vector.tensor_copy(pos_stage[:, rt:rt + 1], idx8[:, 1:2])
                    # keep_neg = (gmax > -0.9), keep_pos = (gmin < -0.9)
                    kn = smalls.tile([128, 1], F32, tag="kn")
                    nc.scalar.activation(kn[:], top8[:],
                                         mybir.ActivationFunctionType.Sign,
                                         scale=1.0, bias=bias09[:])
                    nc.scalar.activation(kn[:], kn[:],
                                         mybir.ActivationFunctionType.Relu,
                                         bias=bias0[:])
                    kp = smalls.tile([128, 1], F32, tag="kp")
                    nc.scalar.activation(kp[:], gmin[:],
                                         mybir.ActivationFunctionType.Sign,
                                         scale=-1.0, bias=biasm09[:])
                    nc.scalar.activation(kp[:], kp[:],
                                         mybir.ActivationFunctionType.Relu,
                                         bias=bias0[:])
                    nc.vector.tensor_tensor(keep_stage[:, rt:rt + 1], kn[:],
                                            kp[:], mybir.AluOpType.mult)

            if use_for_i:
                with tc.For_i(0, k_repeat, 1):
                    main_body()
            else:
                for _ in range(k_repeat):
                    main_body()

            nc.sync.dma_start(neg_out[:], neg_stage[:])
            nc.sync.dma_start(pos_out[:], pos_stage[:])
            nc.sync.dma_start(keep_out[:], keep_stage[:])

    nc.compile()
    return nc


_CACHED_NC = None


def kernel(l_embeds: np.ndarray, l_labels: np.ndarray):
    global _CACHED_NC
    if _CACHED_NC is None:
        _CACHED_NC = build_program()
    nc = _CACHED_NC

    x = np.ascontiguousarray(np.asarray(l_embeds, dtype=np.float32))
    lab_i = np.asarray(l_labels)
    lab = lab_i.astype(np.float32)

    in_maps = []
    for m in range(NCORES):
        sl = slice(m * STRIP, (m + 1) * STRIP)
        in_maps.append({
            "x_full": x,
            "x_strip": np.ascontiguousarray(x[sl]),
            "lab_full": lab.reshape(1, N),
            # lab_strip[p, r] = lab[m*STRIP + r*128 + p]
            "lab_strip": np.ascontiguousarray(
                lab[sl].reshape(RT, 128).T),
        })

    res = run_bass_kernel_spmd(nc, in_maps, list(range(NCORES))).results

    neg = np.empty(N, np.int64)
    pos = np.empty(N, np.int64)
    keep = np.empty(N, np.float32)
    for m in range(NCORES):
        sl = slice(m * STRIP, (m + 1) * STRIP)
        # stage[p, r] -> row r*128+p  =>  transpose to [rt, 128] then flatten
        neg[sl] = res[m]["neg_out"].T.reshape(-1)
        pos[sl] = res[m]["pos_out"].T.reshape(-1)
        keep[sl] = res[m]["keep_out"].T.reshape(-1)

    idt = np.int32 if lab_i.dtype != np.int64 else np.int64
    anchor = np.arange(N, dtype=idt)
    return (anchor, pos.astype(idt), neg.astype(idt), keep > 0.5)

